# revision 1
# baseline (speedup 1.0000x reference)
"""Trainium2 Bass kernel for a dense transformer encoder layer.

Reference computation (per batch b):
    q = x.reshape(L, H, E)                       # H=16 heads, E=64
    scores = q @ q^T per head, scaled softmax    # A = softmax(s/8)
    new_x  = concat_h(A_h @ q_h)                 # [L, D]
    x1 = LN(x + new_x; g1, be1)
    y  = relu(x1 @ w1^T + b1) @ w2^T + b2
    out = LN(x1 + y; g2, be2)

Sharding: pure data parallel over (batch, seq-half): core c handles
batch c//2, query rows [(c%2)*1024, +1024).  Keys/values span the full
sequence of that batch, so every core gets the whole x[b] (queries
reordered first) and the full FFN weights.  No device collectives.

Per-core algorithm (all matmuls fp32r):
  - scores are computed TRANSPOSED ([s, l] layout) so that the softmax'd
    exp(scores^T) can be the moving operand of the AV matmul.
  - softmax denominator comes for free: V is stored interleaved with a
    ones column per head ([s, 65] stationary), so the AV matmul emits
    U^T = [V|1]^T E^T with the rowsum in row 64.
  - U^T tiles are PE-transposed back to [l, 65]; dividing by column 64
    completes the softmax.  No max-subtraction (scaled scores <= ~14,
    exp fits fp32 comfortably).
  - FFN streams w1/w2 from HBM as pre-tiled [128,128] stationaries;
    FFN2 accumulates f-chunks in groups of 8 through PSUM into SBUF.
"""

import numpy as np

import concourse.bass as bass
import concourse.tile as tile
from concourse import bacc
from concourse import mybir
from concourse.masks import make_identity

F32 = mybir.dt.float32
F32R = mybir.dt.float32r
BF16 = mybir.dt.bfloat16
EXP = mybir.ActivationFunctionType.Exp
RELU = mybir.ActivationFunctionType.Relu
SQRT = mybir.ActivationFunctionType.Sqrt
ADD = mybir.AluOpType.add
SUB = mybir.AluOpType.subtract
MUL = mybir.AluOpType.mult

LN_EPS = 1e-5
E = 64          # head dim
W = E + 1       # head dim + ones column
P = 128         # partitions


def r(ap):
    return ap.bitcast(F32R)


def build_program(S=2048, D=1024, F=4096, n_cores_unused=8):
    """Build the per-core Bass program.  S = full seq len, queries are the
    first Lq = S//2 rows of xb."""
    H = D // E
    Lq = S // 2
    ST = S // P          # s-tiles
    LT = Lq // P         # query row tiles
    DT = D // P          # d chunks
    FT = F // P          # f tiles
    FG = 8               # f-tiles per FFN2 accumulation group
    NSL = max(1, Lq // 512)
    SL = Lq // NSL       # l-slab width (moving N); 512 at full size
    assert SL >= 256, "fp32r needs moving free dim >= 256"
    GS = min(512, D)     # bn_stats subgroup size

    nc = bacc.Bacc("TRN2")

    xb = nc.dram_tensor("xb", [S, D], F32, kind="ExternalInput")
    xb16 = nc.dram_tensor("xb16", [S, D], BF16, kind="ExternalInput")
    w1t = nc.dram_tensor("w1t", [DT, FT, P, P], BF16, kind="ExternalInput")
    w2t = nc.dram_tensor("w2t", [FT, DT, P, P], BF16, kind="ExternalInput")
    b1 = nc.dram_tensor("b1", [F], F32, kind="ExternalInput")
    b2 = nc.dram_tensor("b2", [D], F32, kind="ExternalInput")
    g1 = nc.dram_tensor("g1", [D], F32, kind="ExternalInput")
    be1 = nc.dram_tensor("be1", [D], F32, kind="ExternalInput")
    g2 = nc.dram_tensor("g2", [D], F32, kind="ExternalInput")
    be2 = nc.dram_tensor("be2", [D], F32, kind="ExternalInput")
    out = nc.dram_tensor("out", [Lq, D], F32, kind="ExternalOutput")

    def bcast(dram_vec, n):
        a = dram_vec[:]
        return bass.AP(tensor=a.tensor, offset=a.offset, ap=[[0, P]] + a.ap)

    with tile.TileContext(nc) as tc:
        with (
            tc.tile_pool(name="persist", bufs=1) as persist,
            tc.tile_pool(name="small", bufs=6) as small,
            tc.tile_pool(name="gb", bufs=2) as gbp,
        ):
            ident = persist.tile([P, P], F32)
            make_identity(nc, ident)
            ident16 = persist.tile([P, P], BF16)
            make_identity(nc, ident16)
            b1s = persist.tile([P, FT], F32)
            nc.sync.dma_start(out=b1s, in_=b1[:].rearrange("(t p) -> p t", p=P))
            b2s = persist.tile([P, DT], F32)
            nc.sync.dma_start(out=b2s, in_=b2[:].rearrange("(t p) -> p t", p=P))
            epst = persist.tile([P, 1], F32)
            nc.vector.memset(epst, LN_EPS)
            # new_x doubles as x1 (LN1 is computed in place) and as res2.
            new_x = persist.tile([P, LT, D], F32)

            # ---------------- stage 0 + attention ----------------
            with (
                tc.tile_pool(name="attn_sb", bufs=1) as asb,
                tc.tile_pool(name="etp", bufs=3) as etp,
                tc.tile_pool(name="utsp", bufs=16) as utsp,
            ):
                # x interleaved with ones columns: per s-tile [P, H, W]
                vaug = asb.tile([P, ST, H, W], BF16)
                for u in range(ST):
                    nc.sync.dma_start(
                        out=vaug[:, u, :, 0:E],
                        in_=xb16[u * P:(u + 1) * P, :].rearrange(
                            "p (h e) -> p h e", e=E),
                    )
                    nc.vector.memset(vaug[:, u, :, E:W], 1.0)

                # x^T tiles: [P, DT, S]; d-tile t holds heads 2t, 2t+1.
                # Transpose sources must be contiguous-d and transpose
                # outputs must land at PSUM partition 0, so stream a plain
                # copy of xb and transpose 128x128 blocks.
                xT = asb.tile([P, DT, S], BF16)
                with (
                    tc.tile_pool(name="xin", bufs=3) as xinp,
                    tc.tile_pool(name="t0p", bufs=3, space="PSUM") as t0p,
                ):
                    for u in range(ST):
                        xi = xinp.tile([P, D], BF16, tag="xi")
                        nc.sync.dma_start(
                            out=xi, in_=xb16[u * P:(u + 1) * P, :])
                        for t in range(DT):
                            tp = t0p.tile([P, P], BF16)
                            nc.tensor.transpose(
                                tp, xi[:, t * P:(t + 1) * P], ident16)
                            nc.vector.tensor_copy(
                                out=xT[:, t, u * P:(u + 1) * P], in_=tp)

                with (
                    tc.tile_pool(name="scp", bufs=2, space="PSUM") as scp,
                    tc.tile_pool(name="utp", bufs=2, space="PSUM") as utp,
                ):
                    uts_all = []
                    for h in range(H):
                        t, ro = h // 2, (h % 2) * E
                        ut = utp.tile([W, Lq], F32)

                        def emit_ut(et_u):
                            et_p, u_p = et_u
                            for s in range(NSL):
                                nc.tensor.matmul(
                                    ut[:, s * SL:(s + 1) * SL],
                                    vaug[:, u_p, h, :],
                                    et_p[:, s * SL:(s + 1) * SL],
                                    start=(u_p == 0), stop=(u_p == ST - 1))

                        # software pipeline: AV matmul lags one chunk so the
                        # in-order PE can issue scores(u+1) during exp(u)
                        pend = None
                        for u in range(ST):
                            sc = scp.tile([P, Lq], F32)
                            et = etp.tile([P, Lq], BF16)
                            for s in range(NSL):
                                nc.tensor.matmul(
                                    sc[:, s * SL:(s + 1) * SL],
                                    xT[ro:ro + E, t, u * P:(u + 1) * P],
                                    xT[ro:ro + E, t, s * SL:(s + 1) * SL],
                                    start=True, stop=True)
                            if pend is not None:
                                emit_ut(pend)
                            nc.scalar.activation(
                                out=et, in_=sc, func=EXP, scale=1.0 / 8.0)
                            pend = (et, u)
                        emit_ut(pend)
                        uts = utsp.tile([W, Lq], F32)
                        nc.vector.tensor_copy(out=uts, in_=ut)
                        nc.vector.reciprocal(
                            out=uts[E:W, :], in_=uts[E:W, :])
                        uts_all.append(uts)

                # dense U epilogue: transpose U^T tiles and divide by the
                # (already reciprocated) rowsum riding in column 64
                with tc.tile_pool(name="utr2", bufs=6, space="PSUM") as utrp:
                    for h in range(H):
                        for lt in range(LT):
                            up = utrp.tile([P, W], F32)
                            nc.tensor.transpose(
                                up, uts_all[h][:, lt * P:(lt + 1) * P],
                                ident[0:W, 0:W])
                            nc.vector.tensor_scalar_mul(
                                out=new_x[:, lt, h * E:(h + 1) * E],
                                in0=up[:, 0:E], scalar1=up[:, E:W])

                # residual 1 + LN1 (in place on new_x)
                g1b = gbp.tile([P, D], F32, tag="g")
                nc.gpsimd.dma_start(out=g1b, in_=bcast(g1, D))
                be1b = gbp.tile([P, D], F32, tag="be")
                nc.gpsimd.dma_start(out=be1b, in_=bcast(be1, D))
                for lt in range(LT):
                    xq = gbp.tile([P, D], F32, tag="xq")
                    nc.sync.dma_start(
                        out=xq, in_=xb[lt * P:(lt + 1) * P, :])
                    nc.vector.tensor_add(
                        out=new_x[:, lt, :],
                        in0=new_x[:, lt, :],
                        in1=xq)
                    _layer_norm_inplace(nc, small, new_x[:, lt, :],
                                        new_x[:, lt, :], g1b, be1b, epst, GS)

            # ---------------- FFN ----------------
            with (
                tc.tile_pool(name="ffn_sb", bufs=1) as fsb,
                tc.tile_pool(name="htp", bufs=FG + 1) as htp,
                tc.tile_pool(name="w1p", bufs=16) as w1p,
                tc.tile_pool(name="w2p", bufs=16) as w2p,
                tc.tile_pool(name="outp", bufs=3) as outp,
            ):
                x1T = fsb.tile([P, DT, Lq], BF16)
                with tc.tile_pool(name="x1tp", bufs=3, space="PSUM") as x1tp:
                    for lt in range(LT):
                        for c in range(DT):
                            tp = x1tp.tile([P, P], F32)
                            nc.tensor.transpose(
                                tp, new_x[:, lt, c * P:(c + 1) * P], ident)
                            nc.vector.tensor_copy(
                                out=x1T[:, c, lt * P:(lt + 1) * P], in_=tp)

                yT = fsb.tile([P, DT, Lq], F32)
                with (
                    tc.tile_pool(name="hpp", bufs=4, space="PSUM") as hpp,
                    tc.tile_pool(name="ypp", bufs=3, space="PSUM") as ypp,
                ):
                    for fg in range(FT // FG):
                        hts = []
                        for j in range(FG):
                            ft = fg * FG + j
                            ht = htp.tile([P, Lq], BF16, tag="ht")
                            for s in range(NSL):
                                hp = hpp.tile([P, SL], F32)
                                for dc in range(DT):
                                    wt = w1p.tile([P, P], BF16, tag="w1")
                                    nc.sync.dma_start(
                                        out=wt, in_=w1t[dc, ft, :, :])
                                    nc.tensor.matmul(
                                        hp, wt,
                                        x1T[:, dc, s * SL:(s + 1) * SL],
                                        start=(dc == 0), stop=(dc == DT - 1))
                                nc.scalar.activation(
                                    out=ht[:, s * SL:(s + 1) * SL], in_=hp,
                                    func=RELU, bias=b1s[:, ft:ft + 1])
                            hts.append(ht)
                        for dt in range(DT):
                            for s in range(NSL):
                                yp = ypp.tile([P, SL], F32)
                                for j in range(FG):
                                    wt = w2p.tile([P, P], BF16, tag="w2")
                                    nc.sync.dma_start(
                                        out=wt, in_=w2t[fg * FG + j, dt, :, :])
                                    nc.tensor.matmul(
                                        yp, wt,
                                        hts[j][:, s * SL:(s + 1) * SL],
                                        start=(j == 0), stop=(j == FG - 1))
                                ysl = yT[:, dt, s * SL:(s + 1) * SL]
                                if fg == 0:
                                    nc.vector.tensor_scalar_add(
                                        out=ysl, in0=yp,
                                        scalar1=b2s[:, dt:dt + 1])
                                else:
                                    nc.vector.tensor_add(
                                        out=ysl, in0=ysl, in1=yp)

                # residual 2 + LN2 -> out
                g2b = gbp.tile([P, D], F32, tag="g")
                nc.gpsimd.dma_start(out=g2b, in_=bcast(g2, D))
                be2b = gbp.tile([P, D], F32, tag="be")
                nc.gpsimd.dma_start(out=be2b, in_=bcast(be2, D))
                with tc.tile_pool(name="ytp", bufs=3, space="PSUM") as ytp:
                    for lt in range(LT):
                        for dt in range(DT):
                            tp = ytp.tile([P, P], F32)
                            nc.tensor.transpose(
                                tp, yT[:, dt, lt * P:(lt + 1) * P], ident)
                            nc.vector.tensor_add(
                                out=new_x[:, lt, dt * P:(dt + 1) * P],
                                in0=new_x[:, lt, dt * P:(dt + 1) * P],
                                in1=tp)
                        ot = outp.tile([P, D], F32)
                        _layer_norm_inplace(nc, small, ot, new_x[:, lt, :],
                                            g2b, be2b, epst, GS)
                        nc.sync.dma_start(
                            out=out[lt * P:(lt + 1) * P, :], in_=ot)

    nc.finalize()
    return nc


def _layer_norm_inplace(nc, small, out_ap, x_ap, gb, beb, epst, GS):
    """out = (x - mean(x)) * rsqrt(var(x) + eps) * g + be over free dim.
    First two steps are computed in place on x_ap; out_ap may equal x_ap
    or be a fresh tile."""
    D = x_ap.shape[-1]
    ngr = D // GS
    st = small.tile([P, ngr, 6], F32, tag="bnst")
    xg = x_ap.rearrange("p (g k) -> p g k", k=GS)
    for g in range(ngr):
        nc.vector.bn_stats(out=st[:, g, :], in_=xg[:, g, :])
    mv = small.tile([P, 2], F32, tag="bnmv")
    nc.vector.bn_aggr(out=mv, in_=st)
    rstd = small.tile([P, 1], F32, tag="rstd")
    nc.scalar.activation(out=rstd, in_=mv[:, 1:2], func=SQRT, bias=epst)
    nc.vector.reciprocal(out=rstd, in_=rstd)
    nc.vector.tensor_scalar(
        out=x_ap, in0=x_ap, scalar1=mv[:, 0:1], scalar2=rstd,
        op0=SUB, op1=MUL)
    nc.vector.tensor_mul(out=x_ap, in0=x_ap, in1=gb)
    nc.vector.tensor_add(out=out_ap, in0=x_ap, in1=beb)


# ---------------------------------------------------------------------------
# host side
# ---------------------------------------------------------------------------

_PROG_CACHE = {}


def get_program(S=2048, D=1024, F=4096):
    key = (S, D, F)
    if key not in _PROG_CACHE:
        _PROG_CACHE[key] = build_program(S, D, F)
    return _PROG_CACHE[key]


def make_in_maps(x, w1, b1, w2, b2, g1, be1, g2, be2, n_cores=8):
    B, L, D = x.shape
    F = w1.shape[0]
    Lq = L // 2
    DT, FT = D // 128, F // 128
    import ml_dtypes
    w1t = np.ascontiguousarray(
        w1.T.reshape(DT, 128, FT, 128).transpose(0, 2, 1, 3)).astype(ml_dtypes.bfloat16)
    w2t = np.ascontiguousarray(
        w2.T.reshape(FT, 128, DT, 128).transpose(0, 2, 1, 3)).astype(ml_dtypes.bfloat16)
    common = dict(w1t=w1t, w2t=w2t, b1=b1, b2=b2, g1=g1, be1=be1,
                  g2=g2, be2=be2)
    in_maps = []
    for c in range(n_cores):
        b, half = c // 2, c % 2
        lo = half * Lq
        xq = x[b, lo:lo + Lq]
        xo = x[b, Lq - lo:2 * Lq - lo]
        xbl = np.ascontiguousarray(np.concatenate([xq, xo], axis=0))
        in_maps.append(dict(xb=xbl, xb16=xbl.astype(ml_dtypes.bfloat16),
                            **common))
    return in_maps


def kernel(x, w1, b1, w2, b2, g1, be1, g2, be2):
    from concourse.bass_utils import run_bass_kernel_spmd

    x = np.asarray(x, dtype=np.float32)
    B, L, D = x.shape
    F = w1.shape[0]
    Lq = L // 2
    n_cores = 2 * B
    nc = get_program(L, D, F)
    in_maps = make_in_maps(x, np.asarray(w1, np.float32), np.asarray(b1, np.float32),
                           np.asarray(w2, np.float32), np.asarray(b2, np.float32),
                           np.asarray(g1, np.float32), np.asarray(be1, np.float32),
                           np.asarray(g2, np.float32), np.asarray(be2, np.float32),
                           n_cores)
    res = run_bass_kernel_spmd(nc, in_maps, core_ids=list(range(n_cores)))
    outp = np.empty((B, L, D), dtype=np.float32)
    for c in range(n_cores):
        b, half = c // 2, c % 2
        outp[b, half * Lq:(half + 1) * Lq] = res.results[c]["out"]
    return outp



# revision 12
# speedup vs baseline: 2.0383x; 2.0383x over previous
"""Trainium2 Bass kernel for a dense transformer encoder layer.

Reference computation (per batch b):
    q = x.reshape(L, H, E)                       # H=16 heads, E=64
    scores = q @ q^T per head, scaled softmax    # A = softmax(s/8)
    new_x  = concat_h(A_h @ q_h)                 # [L, D]
    x1 = LN(x + new_x; g1, be1)
    y  = relu(x1 @ w1^T + b1) @ w2^T + b2
    out = LN(x1 + y; g2, be2)

Sharding: pure data parallel over (batch, seq-half): core c handles
batch c//2, query rows [(c%2)*1024, +1024).  Keys/values span the full
sequence of that batch, so every core gets the whole x[b] (queries
reordered first) and the full FFN weights.  No device collectives.

v2 design notes (all matmuls bf16, PE does ONLY matmuls):
  - x^T, U^T, x1^T are produced with DMA xbar transposes (16-bit dtype,
    src partition %16, free %128), not PE transposes.
  - scores are computed TRANSPOSED ([s, l]) so exp(scores^T) is the
    moving operand of the AV matmul; V carries a ones column so the
    softmax denominator rides along in row 64 of U^T (rows 65..79 pad
    to the xbar 16-row granularity with zero columns).
  - FFN weights are streamed from HBM once, as one [128, 1024] stripe
    per 128-row block (64 DMAs total instead of 1024 tile DMAs).
  - FFN1 accumulates over d-chunks with the stationary w1 tile reused
    across both 512-wide moving slabs; FFN2 uses h^T tiles as the
    stationary and w2 stripes as the moving operand, producing y
    ROW-major directly into PSUM (no output transpose at all).
"""

import numpy as np

import concourse.bass as bass
import concourse.tile as tile
from concourse import bacc
from concourse import mybir

F32 = mybir.dt.float32
BF16 = mybir.dt.bfloat16
EXP = mybir.ActivationFunctionType.Exp
RELU = mybir.ActivationFunctionType.Relu
SQRT = mybir.ActivationFunctionType.Sqrt
ADD = mybir.AluOpType.add
SUB = mybir.AluOpType.subtract
MUL = mybir.AluOpType.mult

LN_EPS = 1e-5
E = 64          # head dim
W = 80          # head dim + ones column + pad to xbar 16-row granularity
P = 128         # partitions


def build_program(S=2048, D=1024, F=4096):
    """Per-core program.  S = full seq len; queries are rows [0, Lq)."""
    H = D // E
    Lq = S // 2
    ST = S // P          # key tiles
    LT = Lq // P         # query row tiles
    DT = D // P          # d chunks
    FT = F // P          # f chunks
    NSL = 2
    SL = Lq // NSL       # moving slab width (512)
    GS = min(512, D)     # bn_stats subgroup size

    nc = bacc.Bacc("TRN2")

    xb = nc.dram_tensor("xb", [S, D], F32, kind="ExternalInput")
    xb16 = nc.dram_tensor("xb16", [S, D], BF16, kind="ExternalInput")
    w1s = nc.dram_tensor("w1s", [FT, P, D], BF16, kind="ExternalInput")
    w2s = nc.dram_tensor("w2s", [FT, P, D], BF16, kind="ExternalInput")
    b1 = nc.dram_tensor("b1", [F], F32, kind="ExternalInput")
    b2 = nc.dram_tensor("b2", [D], F32, kind="ExternalInput")
    g1 = nc.dram_tensor("g1", [D], F32, kind="ExternalInput")
    be1 = nc.dram_tensor("be1", [D], F32, kind="ExternalInput")
    g2 = nc.dram_tensor("g2", [D], F32, kind="ExternalInput")
    be2 = nc.dram_tensor("be2", [D], F32, kind="ExternalInput")
    out = nc.dram_tensor("out", [Lq, D], F32, kind="ExternalOutput")

    def bcast(dram_vec, n):
        a = dram_vec[:]
        return bass.AP(tensor=a.tensor, offset=a.offset, ap=[[0, P]] + a.ap)

    with tile.TileContext(nc) as tc:
        with (
            tc.tile_pool(name="persist", bufs=1) as persist,
            tc.tile_pool(name="small", bufs=6) as small,
            tc.tile_pool(name="gb", bufs=1) as gbp,
            tc.tile_pool(name="resp", bufs=3) as resp,
        ):
            b1s = persist.tile([P, FT], F32)
            nc.sync.dma_start(out=b1s, in_=b1[:].rearrange("(t p) -> p t", p=P))
            epst = persist.tile([P, 1], F32)
            nc.vector.memset(epst, LN_EPS)
            # x1 (post-LN1) in bf16: residual-2 source and FFN1 input
            x1b = persist.tile([P, LT, D], BF16)
            # x1^T: [p, lt, dc, j] = x1[lt*128+j, dc*128+p]
            x1T = persist.tile([P, LT, DT, P], BF16)

            # ---------------- attention ----------------
            with (
                tc.tile_pool(name="attn_sb", bufs=1) as asb,
                tc.tile_pool(name="xrp", bufs=3) as xrp,
                tc.tile_pool(name="etp", bufs=4) as etp,
                tc.tile_pool(name="utsp", bufs=2) as utsp,
                tc.tile_pool(name="usp", bufs=3) as usp,
                tc.tile_pool(name="recp", bufs=4) as recp,
            ):
                # attention output, bf16 (residual add upcasts later)
                new_x = asb.tile([P, LT, D], BF16)
                # x^T tiles: [P, DT, S]; d-chunk t holds heads 2t, 2t+1
                xT = asb.tile([P, DT, S], BF16)
                for t in range(DT):
                    nc.sync.dma_start_transpose(
                        out=xT[:, t, :], in_=xb16[:, t * P:(t + 1) * P])

                # V interleaved with ones column + zero pad: [P, ST, H, W]
                vaug = asb.tile([P, ST, H, W], BF16)
                nc.gpsimd.memset(vaug[:, :, :, E:W], 0.0)
                nc.gpsimd.memset(vaug[:, :, :, E:E + 1], 1.0)
                for u in range(ST):
                    xr = xrp.tile([P, D], BF16, tag="xr")
                    nc.sync.dma_start(out=xr, in_=xb16[u * P:(u + 1) * P, :])
                    nc.vector.tensor_copy(
                        out=vaug[:, u, :, 0:E],
                        in_=xr.rearrange("p (h e) -> p h e", e=E))

                # queries (f32 rows) for the LN1 residual
                xq = asb.tile([P, LT, D], F32)
                for lt in range(LT):
                    nc.sync.dma_start(
                        out=xq[:, lt, :], in_=xb[lt * P:(lt + 1) * P, :])

                g1b = gbp.tile([P, D], F32, tag="g")
                nc.gpsimd.dma_start(out=g1b, in_=bcast(g1, D))
                be1b = gbp.tile([P, D], F32, tag="be")
                nc.gpsimd.dma_start(out=be1b, in_=bcast(be1, D))

                with (
                    tc.tile_pool(name="scp", bufs=2, space="PSUM") as scp,
                    tc.tile_pool(name="utp", bufs=2, space="PSUM") as utp,
                ):
                    for h in range(H):
                        t, ro = h // 2, (h % 2) * E
                        ut = utp.tile([W, Lq], F32)

                        def emit_ut(et_u, ut=ut, h=h):
                            et_p, u_p = et_u
                            for s in range(NSL):
                                nc.tensor.matmul(
                                    ut[:, s * SL:(s + 1) * SL],
                                    vaug[:, u_p, h, :],
                                    et_p[:, s * SL:(s + 1) * SL],
                                    start=(u_p == 0), stop=(u_p == ST - 1))

                        # software pipeline: AV lags TWO chunks so every PE
                        # instruction's exp input is long done -> the PE
                        # issues back-to-back with no semaphore stalls
                        pend = []
                        for u in range(ST):
                            sc = scp.tile([P, Lq], F32)
                            et = etp.tile([P, Lq], BF16)
                            for s in range(NSL):
                                nc.tensor.matmul(
                                    sc[:, s * SL:(s + 1) * SL],
                                    xT[ro:ro + E, t, u * P:(u + 1) * P],
                                    xT[ro:ro + E, t, s * SL:(s + 1) * SL],
                                    start=True, stop=True)
                            if len(pend) >= 2:
                                emit_ut(pend.pop(0))
                            nc.scalar.activation(
                                out=et, in_=sc, func=EXP, scale=1.0 / 8.0)
                            pend.append((et, u))
                        for p_ in pend:
                            emit_ut(p_)

                        uts = utsp.tile([W, Lq], BF16)
                        nc.vector.tensor_copy(out=uts, in_=ut)
                        # U: [p, lt, w] = U^T[w, lt*128+p]
                        us = usp.tile([P, LT, W], BF16)
                        nc.sync.dma_start_transpose(out=us, in_=uts)
                        rec = recp.tile([P, LT], F32)
                        nc.vector.reciprocal(out=rec, in_=us[:, :, E])
                        for lt in range(LT):
                            nc.vector.tensor_scalar_mul(
                                out=new_x[:, lt, h * E:(h + 1) * E],
                                in0=us[:, lt, 0:E],
                                scalar1=rec[:, lt:lt + 1])

                # residual 1 + LN1 -> x1b (bf16) and x1T (dma transpose)
                for lt in range(LT):
                    xs = resp.tile([P, D], F32, tag="res")
                    nc.vector.tensor_add(
                        out=xs, in0=new_x[:, lt, :], in1=xq[:, lt, :])
                    _layer_norm(nc, small, x1b[:, lt, :], xs,
                                g1b, be1b, epst, GS)
                    nc.sync.dma_start_transpose(
                        out=x1T[:, lt, :, :], in_=x1b[:, lt, :])

            # ---------------- FFN ----------------
            with (
                tc.tile_pool(name="ffn_sb", bufs=1) as fsb,
                tc.tile_pool(name="w1p", bufs=3) as w1p,
                tc.tile_pool(name="otp", bufs=2) as otp,
            ):
                g2b = gbp.tile([P, D], F32, tag="g")
                nc.gpsimd.dma_start(out=g2b, in_=bcast(g2, D))
                be2b = gbp.tile([P, D], F32, tag="be")
                nc.gpsimd.dma_start(out=be2b, in_=bcast(be2, D))
                b2b = gbp.tile([P, D], F32, tag="b2")
                nc.gpsimd.dma_start(out=b2b, in_=bcast(b2, D))

                # all w2 stripes + all h^T tiles stay resident
                w2a = fsb.tile([P, FT, D], BF16)
                for j in range(FT):
                    nc.sync.dma_start(out=w2a[:, j, :], in_=w2s[j])
                hts = fsb.tile([P, FT, Lq], BF16)

                # FFN1: h^T[f, l] = relu(w1 x1^T + b1)
                with tc.tile_pool(name="hpp", bufs=4, space="PSUM") as hpp:
                    for ft in range(FT):
                        wt = w1p.tile([P, D], BF16, tag="w1")
                        nc.sync.dma_start(out=wt, in_=w1s[ft])
                        hp = [hpp.tile([P, SL], F32, name=f"hp{s}",
                                       tag=f"hp{s}")
                              for s in range(NSL)]
                        # first two f-chunks: finish slab 0 (query rows
                        # 0..511) before touching slab 1, so FFN1 starts as
                        # soon as LN1 of the first 4 row tiles lands
                        if ft < 2:
                            loop = [(s, dc) for s in range(NSL)
                                    for dc in range(DT)]
                        else:
                            loop = [(s, dc) for dc in range(DT)
                                    for s in range(NSL)]
                        for s, dc in loop:
                            nc.tensor.matmul(
                                hp[s],
                                wt[:, dc * P:(dc + 1) * P],
                                x1T[:, s * (LT // NSL):(s + 1) * (LT // NSL), dc, :],
                                start=(dc == 0), stop=(dc == DT - 1))
                        for s in range(NSL):
                            nc.scalar.activation(
                                out=hts[:, ft, s * SL:(s + 1) * SL],
                                in_=hp[s], func=RELU,
                                bias=b1s[:, ft:ft + 1])

                # FFN2: y[l, d] = sum_j (h^T_j)^T w2_j  (row-major output)
                with tc.tile_pool(name="ypp", bufs=2, space="PSUM") as ypp:
                    for lt in range(LT):
                        yp = ypp.tile([P, D], F32)
                        for j in range(FT):
                            for s in range(NSL):
                                nc.tensor.matmul(
                                    yp[:, s * SL:(s + 1) * SL],
                                    hts[:, j, lt * P:(lt + 1) * P],
                                    w2a[:, j, s * SL:(s + 1) * SL],
                                    start=(j == 0), stop=(j == FT - 1))
                        # residual 2 + b2 + LN2 -> out
                        xs = resp.tile([P, D], F32, tag="res")
                        nc.vector.scalar_tensor_tensor(
                            out=xs, in0=yp, scalar=1.0,
                            in1=x1b[:, lt, :], op0=MUL, op1=ADD)
                        nc.vector.tensor_add(out=xs, in0=xs, in1=b2b)
                        ot = otp.tile([P, D], F32, tag="ot")
                        _layer_norm(nc, small, ot, xs, g2b, be2b, epst, GS)
                        nc.sync.dma_start(
                            out=out[lt * P:(lt + 1) * P, :], in_=ot)

    nc.finalize()
    return nc


def _layer_norm(nc, small, out_ap, x_ap, gb, beb, epst, GS):
    """out = (x - mean(x)) * rsqrt(var(x) + eps) * g + be over free dim.
    x_ap is clobbered (normalized in place); out_ap gets the final value
    and may have a different dtype."""
    D = x_ap.shape[-1]
    ngr = D // GS
    st = small.tile([P, ngr, 6], F32, tag="bnst")
    xg = x_ap.rearrange("p (g k) -> p g k", k=GS)
    for g in range(ngr):
        nc.vector.bn_stats(out=st[:, g, :], in_=xg[:, g, :])
    mv = small.tile([P, 2], F32, tag="bnmv")
    nc.vector.bn_aggr(out=mv, in_=st)
    rstd = small.tile([P, 1], F32, tag="rstd")
    nc.scalar.activation(out=rstd, in_=mv[:, 1:2], func=SQRT, bias=epst)
    nc.vector.reciprocal(out=rstd, in_=rstd)
    nc.vector.tensor_scalar(
        out=x_ap, in0=x_ap, scalar1=mv[:, 0:1], scalar2=rstd,
        op0=SUB, op1=MUL)
    nc.vector.tensor_mul(out=x_ap, in0=x_ap, in1=gb)
    nc.vector.tensor_add(out=out_ap, in0=x_ap, in1=beb)


# ---------------------------------------------------------------------------
# host side
# ---------------------------------------------------------------------------

_PROG_CACHE = {}


def get_program(S=2048, D=1024, F=4096):
    key = (S, D, F)
    if key not in _PROG_CACHE:
        _PROG_CACHE[key] = build_program(S, D, F)
    return _PROG_CACHE[key]


def make_in_maps(x, w1, b1, w2, b2, g1, be1, g2, be2, n_cores=8):
    B, L, D = x.shape
    F = w1.shape[0]
    Lq = L // 2
    DT, FT = D // 128, F // 128
    import ml_dtypes
    # w1s[ft, p, dc*128+f] = w1[ft*128+f, dc*128+p]
    w1s = np.ascontiguousarray(
        w1.reshape(FT, 128, DT, 128).transpose(0, 3, 2, 1)
        .reshape(FT, 128, D)).astype(ml_dtypes.bfloat16)
    # w2s[j, p, d] = w2[d, j*128+p]
    w2s = np.ascontiguousarray(
        w2.T.reshape(FT, 128, D)).astype(ml_dtypes.bfloat16)
    common = dict(w1s=w1s, w2s=w2s, b1=b1, b2=b2, g1=g1, be1=be1,
                  g2=g2, be2=be2)
    in_maps = []
    for c in range(n_cores):
        b, half = c // 2, c % 2
        lo = half * Lq
        xq = x[b, lo:lo + Lq]
        xo = x[b, Lq - lo:2 * Lq - lo]
        xbl = np.ascontiguousarray(np.concatenate([xq, xo], axis=0))
        in_maps.append(dict(xb=xbl, xb16=xbl.astype(ml_dtypes.bfloat16),
                            **common))
    return in_maps


def kernel(x, w1, b1, w2, b2, g1, be1, g2, be2):
    from concourse.bass_utils import run_bass_kernel_spmd

    x = np.asarray(x, dtype=np.float32)
    B, L, D = x.shape
    F = w1.shape[0]
    Lq = L // 2
    n_cores = 2 * B
    nc = get_program(L, D, F)
    in_maps = make_in_maps(x, np.asarray(w1, np.float32), np.asarray(b1, np.float32),
                           np.asarray(w2, np.float32), np.asarray(b2, np.float32),
                           np.asarray(g1, np.float32), np.asarray(be1, np.float32),
                           np.asarray(g2, np.float32), np.asarray(be2, np.float32),
                           n_cores)
    res = run_bass_kernel_spmd(nc, in_maps, core_ids=list(range(n_cores)))
    outp = np.empty((B, L, D), dtype=np.float32)
    for c in range(n_cores):
        b, half = c // 2, c % 2
        outp[b, half * Lq:(half + 1) * Lq] = res.results[c]["out"]
    return outp
